# revision 17
# baseline (speedup 1.0000x reference)
"""Trainium2 Bass kernel for the AttentionLayer problem.

Math (per batch):
    Q = inp_q @ Wq + bq            [S, d]
    K = inp_k @ Wk + bk            [S, d]
    V = inp_v @ Wv + bv            [S, d]
    sc = Q @ K^T / sqrt(d)         [Sq, Sk]
    S_ = softmax(sc, axis=0)       (over the QUERY axis)
    H = S_ @ V                     [Sq, d]

Device-side layout strategy (per core, 2 batches):
  * Host feeds transposed activations xT = x^T [D, S] so every matmul
    contracts over the SBUF partition dim with zero on-chip transposes
    of the big activations.
  * Projections produce QT/KT/VT in [d, S] layout (d = 128 partitions).
  * scores^T [k, q] = (KT-slice)^T @ QT, so softmax-over-q is a
    free-axis row reduction: one ACT pass does exp(scale*x) and the
    row sum Z[k].  No max-subtraction is needed: |sc/sqrt(d)| <~ 6 for
    randn inputs, exp() is exact in f32 there.
  * Normalization is folded into V: vs[k, :] = V[k, :] / Z[k], then
    H^T [d, q] += vs-slice^T @ P^T accumulates over k-chunks in PSUM.
  * Host un-transposes H^T -> H.
Compute dtype bf16 (f32 PSUM accumulate), stats in f32.
"""

import math
import sys

sys.path.insert(0, "/opt/trn_rl_repo")

import numpy as np

import concourse.bass as bass  # noqa: E402
import concourse.tile as tile  # noqa: E402
from concourse import bacc, mybir  # noqa: E402
from concourse.masks import make_identity  # noqa: E402

P = 128          # partitions / head dim d
S = 2048         # sequence length
D = 1024         # model dim
DC = D // P      # D chunks (8)
KC = S // P      # key chunks (16)
B_LOC = 2        # batches per core
N_CORES = 8
SCALE = 1.0 / math.sqrt(P)

F32 = mybir.dt.float32
BF16 = mybir.dt.bfloat16

_BUILT = None  # cached (nc,) so repeated kernel() calls reuse the NEFF


def build():
    nc = bacc.Bacc("TRN2", target_bir_lowering=False, debug=False,
                   num_devices=N_CORES)

    dr_in = {}
    for t in ("q", "k", "v"):
        dr_in[t] = nc.dram_tensor(f"{t}T", [B_LOC, D, S], F32,
                                  kind="ExternalInput")
    dr_w = {t: nc.dram_tensor(f"w{t}", [D, P], F32, kind="ExternalInput")
            for t in ("q", "k", "v")}
    dr_b = {t: nc.dram_tensor(f"b{t}", [P], F32, kind="ExternalInput")
            for t in ("q", "k", "v")}
    dr_out = nc.dram_tensor("out", [B_LOC, P, S], F32, kind="ExternalOutput")

    with tile.TileContext(nc) as tc:
        with (
            tc.tile_pool(name="const", bufs=1) as const,
            tc.tile_pool(name="stream", bufs=10) as stream,
            tc.tile_pool(name="proj", bufs=2) as proj,
            tc.tile_pool(name="kctp", bufs=10) as kctp,
            tc.tile_pool(name="ptp", bufs=18) as ptp,
            tc.tile_pool(name="stats", bufs=4) as stats,
            tc.tile_pool(name="recp", bufs=36) as recp,
            tc.tile_pool(name="osb", bufs=2) as osb,
            tc.tile_pool(name="ps_big", bufs=2, space="PSUM") as ps_big,
            tc.tile_pool(name="ps_acc", bufs=1, space="PSUM") as ps_acc,
        ):
            # ---- constants ----
            w_sb = {}
            b_sb = {}
            for t in ("q", "k", "v"):
                w_sb[t] = const.tile([P, DC, P], BF16, tag=f"w{t}", name=f"w{t}")
                # [D, P] -> [p, c, e]; cast f32->bf16 inline (SWDGE)
                nc.gpsimd.dma_start(
                    w_sb[t][:],
                    dr_w[t].ap().rearrange("(c p) e -> p c e", p=P))
                b_sb[t] = const.tile([P, 1], F32, tag=f"b{t}", name=f"b{t}")
                nc.sync.dma_start(
                    b_sb[t][:],
                    dr_b[t].ap().rearrange("(p o) -> p o", o=1))
            ident = const.tile([P, P], BF16, tag="ident", name="ident")
            make_identity(nc, ident[:])

            def emit_qt(b):
                """Q projection: 8 full-S chunks -> qt [d, S] bf16."""
                halves = [ps_big.tile([P, 1024], F32, tag="big", name="q_ps")
                          for _ in range(2)]
                for c in range(DC):
                    x = stream.tile([P, S], BF16, tag="stream", name="x")
                    nc.gpsimd.dma_start(
                        x[:], dr_in["q"].ap()[b, c * P:(c + 1) * P, :])
                    for h in range(2):
                        for s2 in range(2):
                            nc.tensor.matmul(
                                halves[h][:, s2 * 512:(s2 + 1) * 512],
                                lhsT=w_sb["q"][:, c, :],
                                rhs=x[:, h * 1024 + s2 * 512:
                                      h * 1024 + (s2 + 1) * 512],
                                start=(c == 0), stop=(c == DC - 1))
                qt = proj.tile([P, S], BF16, tag="qT", name="qT")
                for h in range(2):
                    nc.vector.tensor_scalar_add(
                        qt[:, h * 1024:(h + 1) * 1024],
                        halves[h][:], b_sb["q"][:])
                return qt

            def emit_vt_chunk(b, vt_ps, c):
                """One D-chunk of the V projection into vt_ps ("acc")."""
                x = stream.tile([P, S], BF16, tag="stream", name="x")
                nc.gpsimd.dma_start(
                    x[:], dr_in["v"].ap()[b, c * P:(c + 1) * P, :])
                for r in range(4):
                    nc.tensor.matmul(
                        vt_ps[:, r * 512:(r + 1) * 512],
                        lhsT=w_sb["v"][:, c, :],
                        rhs=x[:, r * 512:(r + 1) * 512],
                        start=(c == 0), stop=(c == DC - 1))

            def emit_v_finish(vt_ps):
                """Bias-add, PE-transpose VT -> V [S, d]."""
                vt_sb = proj.tile([P, S], BF16, tag="vT", name="vT")
                for h in range(2):
                    nc.vector.tensor_scalar_add(
                        vt_sb[:, h * 1024:(h + 1) * 1024],
                        vt_ps[:, h * 1024:(h + 1) * 1024], b_sb["v"][:])
                v_sb = proj.tile([P, KC, P], BF16, tag="v", name="v")
                trt = ps_acc.tile([P, S], BF16, tag="acc", name="tr_ps")
                for idx in range(KC):
                    nc.tensor.transpose(
                        trt[:, idx * P:(idx + 1) * P],
                        vt_sb[:, idx * P:(idx + 1) * P], ident[:])
                for g in range(2):
                    nc.vector.tensor_copy(
                        v_sb[:, g * 8:(g + 1) * 8, :],
                        trt[:, g * 1024:(g + 1) * 1024].rearrange(
                            "p (a e) -> p a e", a=8))
                return v_sb

            def emit_kchunk(b, sl):
                """K super-chunk: [D, 256] slab -> kct [d, 256] bf16
                (2 k-chunks worth of KT), so scores start on the first
                slab instead of after the whole K projection."""
                xk = stream.tile([P, DC, 256], BF16, tag="stream",
                                 name="xk")
                nc.gpsimd.dma_start(
                    xk[:],
                    dr_in["k"].ap()[b, :, sl * 256:(sl + 1) * 256]
                    .rearrange("(c p) s -> p c s", p=P))
                kps = ps_big.tile([P, 256], F32, tag="big", name="k_ps")
                for c in range(DC):
                    nc.tensor.matmul(
                        kps[:], lhsT=w_sb["k"][:, c, :], rhs=xk[:, c, :],
                        start=(c == 0), stop=(c == DC - 1))
                kct = kctp.tile([P, 256], BF16, tag="kt", name="kct")
                nc.vector.tensor_scalar_add(kct[:], kps[:], b_sb["k"][:])
                return kct

            def emit_scores(qt, lhsT_ap):
                """One k-chunk of scores^T + exp + 1/Z."""
                pt = ptp.tile([P, S], BF16, tag="pt", name="pt")
                zz = stats.tile([P, 2], F32, tag="z", name="zz")
                for h in range(2):
                    sc = ps_big.tile([P, 1024], F32, tag="big",
                                     name="sc_ps")
                    for s2 in range(2):
                        nc.tensor.matmul(
                            sc[:, s2 * 512:(s2 + 1) * 512],
                            lhsT=lhsT_ap,
                            rhs=qt[:, h * 1024 + s2 * 512:
                                   h * 1024 + (s2 + 1) * 512],
                            start=True, stop=True)
                    nc.scalar.activation(
                        pt[:, h * 1024:(h + 1) * 1024], sc[:],
                        func=mybir.ActivationFunctionType.Exp,
                        scale=SCALE, accum_out=zz[:, h:h + 1])
                rec = recp.tile([P, 1], F32, tag="rec", name="rec")
                nc.vector.tensor_reduce(
                    rec[:], zz[:], axis=mybir.AxisListType.X,
                    op=mybir.AluOpType.add)
                nc.vector.reciprocal(rec[:], rec[:])
                return pt, rec

            def emit_ht_mm(ht, v_sb, pt, rec, kc):
                vs = stats.tile([P, P], BF16, tag="vs", name="vs")
                nc.vector.tensor_scalar_mul(vs[:], v_sb[:, kc, :], rec[:])
                for st in range(4):
                    nc.tensor.matmul(
                        ht[:, st * 512:(st + 1) * 512],
                        lhsT=vs[:],
                        rhs=pt[:, st * 512:(st + 1) * 512],
                        start=(kc == 0), stop=(kc == KC - 1))

            def emit_out(b, ht):
                out_sb = osb.tile([P, S], F32, tag="osb", name="out_sb")
                nc.vector.tensor_copy(out_sb[:], ht[:])
                nc.sync.dma_start(dr_out.ap()[b], out_sb[:])

            # ---- batch 0: DMA order q, k, v with the whole-K projection
            # (its exp chain start is DMA-bound either way, and keeping
            # the "big" slots' sc double-buffer pure avoids per-kc
            # bubbles).  V projects into "acc" while scores run; HT at
            # the end. ----
            qt0 = emit_qt(0)
            khalves = [ps_big.tile([P, 1024], F32, tag="big", name="k_ps")
                       for _ in range(2)]
            for c in range(DC):
                x = stream.tile([P, S], BF16, tag="stream", name="x")
                nc.gpsimd.dma_start(
                    x[:], dr_in["k"].ap()[0, c * P:(c + 1) * P, :])
                for h in range(2):
                    for s2 in range(2):
                        nc.tensor.matmul(
                            khalves[h][:, s2 * 512:(s2 + 1) * 512],
                            lhsT=w_sb["k"][:, c, :],
                            rhs=x[:, h * 1024 + s2 * 512:
                                  h * 1024 + (s2 + 1) * 512],
                            start=(c == 0), stop=(c == DC - 1))
            kt0 = proj.tile([P, S], BF16, tag="kT", name="kT")
            for h in range(2):
                nc.vector.tensor_scalar_add(
                    kt0[:, h * 1024:(h + 1) * 1024],
                    khalves[h][:], b_sb["k"][:])
            # Scores chain for batch 0, with batch-0's V-projection
            # D-chunks interleaved at kc = c+3 — timed so each v-chunk
            # has landed by the time PE reaches its matmuls, and its
            # stream slot frees early enough for batch-1's q DMAs.
            vt_ps0 = ps_acc.tile([P, S], F32, tag="acc", name="vt_ps")
            pts0 = []
            recs0 = []
            for kc in range(KC):
                pt, rec = emit_scores(qt0, kt0[:, kc * P:(kc + 1) * P])
                pts0.append(pt)
                recs0.append(rec)
                if 3 <= kc <= 10:
                    emit_vt_chunk(0, vt_ps0, kc - 3)
            v0 = emit_v_finish(vt_ps0)
            ht0 = ps_acc.tile([P, S], F32, tag="acc", name="ht")

            # ---- batch 1: DMA order q, k, v.  exp starts once q + the
            # first K slab have landed (~2/3 of total DMA) instead of
            # after v.  Batch-0's HT matmuls interleave into the first
            # half of this loop (they only need already-computed data,
            # so they fill PE time while ACT paces the exp chain), and
            # batch-1's own HT runs at the end when its V lands. ----
            qt1 = emit_qt(1)
            pts1 = []
            recs1 = []
            for sl in range(8):
                kct = emit_kchunk(1, sl)
                for j in range(2):
                    kc = 2 * sl + j
                    pt, rec = emit_scores(qt1, kct[:, j * P:(j + 1) * P])
                    pts1.append(pt)
                    recs1.append(rec)
                    emit_ht_mm(ht0, v0, pts0[kc], recs0[kc], kc)
                if sl == 7:
                    emit_out(0, ht0)
            v1_ps = ps_acc.tile([P, S], F32, tag="acc", name="vt_ps")
            for c in range(DC):
                emit_vt_chunk(1, v1_ps, c)
            v1 = emit_v_finish(v1_ps)
            ht1 = ps_acc.tile([P, S], F32, tag="acc", name="ht")
            for kc in range(KC):
                emit_ht_mm(ht1, v1, pts1[kc], recs1[kc], kc)
            emit_out(1, ht1)

    nc.compile()
    return nc


def _get_nc():
    global _BUILT
    if _BUILT is None:
        _BUILT = build()
    return _BUILT


def kernel(inp_q, inp_k, inp_v, Wq_kernel, Wq_bias, Wk_kernel, Wk_bias,
           Wv_kernel, Wv_bias):
    from concourse.bass_utils import run_bass_kernel_spmd

    nc = _get_nc()

    inp = {"q": np.asarray(inp_q, dtype=np.float32),
           "k": np.asarray(inp_k, dtype=np.float32),
           "v": np.asarray(inp_v, dtype=np.float32)}
    w = {"q": np.ascontiguousarray(np.asarray(Wq_kernel, dtype=np.float32)),
         "k": np.ascontiguousarray(np.asarray(Wk_kernel, dtype=np.float32)),
         "v": np.ascontiguousarray(np.asarray(Wv_kernel, dtype=np.float32))}
    bias = {"q": np.ascontiguousarray(np.asarray(Wq_bias, dtype=np.float32)),
            "k": np.ascontiguousarray(np.asarray(Wk_bias, dtype=np.float32)),
            "v": np.ascontiguousarray(np.asarray(Wv_bias, dtype=np.float32))}

    in_maps = []
    for c in range(N_CORES):
        m = {}
        for t in ("q", "k", "v"):
            # [2, S, D] -> [2, D, S] contiguous (pure layout marshalling)
            m[f"{t}T"] = np.ascontiguousarray(
                inp[t][c * B_LOC:(c + 1) * B_LOC].transpose(0, 2, 1))
            m[f"w{t}"] = w[t]
            m[f"b{t}"] = bias[t]
        in_maps.append(m)

    res = run_bass_kernel_spmd(nc, in_maps, list(range(N_CORES)))

    out = np.empty((N_CORES * B_LOC, S, P), dtype=np.float32)
    for c in range(N_CORES):
        # [2, P, S] -> [2, S, P]
        out[c * B_LOC:(c + 1) * B_LOC] = (
            res.results[c]["out"].transpose(0, 2, 1))
    return out


# revision 19
# speedup vs baseline: 1.0709x; 1.0709x over previous
"""Trainium2 Bass kernel for the AttentionLayer problem.

Math (per batch):
    Q = inp_q @ Wq + bq            [S, d]
    K = inp_k @ Wk + bk            [S, d]
    V = inp_v @ Wv + bv            [S, d]
    sc = Q @ K^T / sqrt(d)         [Sq, Sk]
    S_ = softmax(sc, axis=0)       (over the QUERY axis)
    H = S_ @ V                     [Sq, d]

Device-side layout strategy (per core, 2 batches):
  * Host feeds transposed activations xT = x^T [D, S] so every matmul
    contracts over the SBUF partition dim with zero on-chip transposes
    of the big activations.
  * Projections produce QT/KT/VT in [d, S] layout (d = 128 partitions).
  * scores^T [k, q] = (KT-slice)^T @ QT, so softmax-over-q is a
    free-axis row reduction: one ACT pass does exp(scale*x) and the
    row sum Z[k].  No max-subtraction is needed: |sc/sqrt(d)| <~ 6 for
    randn inputs, exp() is exact in f32 there.
  * Normalization is folded into V: vs[k, :] = V[k, :] / Z[k], then
    H^T [d, q] += vs-slice^T @ P^T accumulates over k-chunks in PSUM.
  * Host un-transposes H^T -> H.
Compute dtype bf16 (f32 PSUM accumulate), stats in f32.
"""

import math
import sys

sys.path.insert(0, "/opt/trn_rl_repo")

import numpy as np

import concourse.bass as bass  # noqa: E402
import concourse.tile as tile  # noqa: E402
from concourse import bacc, mybir  # noqa: E402
from concourse.masks import make_identity  # noqa: E402

P = 128          # partitions / head dim d
S = 2048         # sequence length
D = 1024         # model dim
DC = D // P      # D chunks (8)
KC = S // P      # key chunks (16)
B_LOC = 2        # batches per core
N_CORES = 8
SCALE = 1.0 / math.sqrt(P)

F32 = mybir.dt.float32
BF16 = mybir.dt.bfloat16

_BUILT = None  # cached (nc,) so repeated kernel() calls reuse the NEFF


def build():
    nc = bacc.Bacc("TRN2", target_bir_lowering=False, debug=False,
                   num_devices=N_CORES)

    dr_in = {}
    for t in ("q", "k", "v"):
        dr_in[t] = nc.dram_tensor(f"{t}T", [B_LOC, D, S], F32,
                                  kind="ExternalInput")
    dr_w = {t: nc.dram_tensor(f"w{t}", [D, P], F32, kind="ExternalInput")
            for t in ("q", "k", "v")}
    dr_b = {t: nc.dram_tensor(f"b{t}", [P], F32, kind="ExternalInput")
            for t in ("q", "k", "v")}
    dr_out = nc.dram_tensor("out", [B_LOC, P, S], F32, kind="ExternalOutput")

    with tile.TileContext(nc) as tc:
        with (
            tc.tile_pool(name="const", bufs=1) as const,
            tc.tile_pool(name="stream", bufs=12) as stream,
            tc.tile_pool(name="proj", bufs=2) as proj,
            tc.tile_pool(name="kctp", bufs=10) as kctp,
            tc.tile_pool(name="ptp", bufs=14) as ptp,
            tc.tile_pool(name="stats", bufs=4) as stats,
            tc.tile_pool(name="recp", bufs=18) as recp,
            tc.tile_pool(name="osb", bufs=2) as osb,
            tc.tile_pool(name="ps_big", bufs=2, space="PSUM") as ps_big,
            tc.tile_pool(name="ps_acc", bufs=1, space="PSUM") as ps_acc,
        ):
            # ---- constants ----
            w_sb = {}
            b_sb = {}
            for t in ("q", "k", "v"):
                w_sb[t] = const.tile([P, DC, P], BF16, tag=f"w{t}", name=f"w{t}")
                # [D, P] -> [p, c, e]; cast f32->bf16 inline (SWDGE)
                nc.gpsimd.dma_start(
                    w_sb[t][:],
                    dr_w[t].ap().rearrange("(c p) e -> p c e", p=P))
                b_sb[t] = const.tile([P, 1], F32, tag=f"b{t}", name=f"b{t}")
                nc.sync.dma_start(
                    b_sb[t][:],
                    dr_b[t].ap().rearrange("(p o) -> p o", o=1))
            ident = const.tile([P, P], BF16, tag="ident", name="ident")
            make_identity(nc, ident[:])

            def emit_qt(b):
                """Q projection: 8 full-S chunks -> qt [d, S] bf16."""
                halves = [ps_big.tile([P, 1024], F32, tag="big", name="q_ps")
                          for _ in range(2)]
                for c in range(DC):
                    x = stream.tile([P, S], BF16, tag="stream", name="x")
                    nc.gpsimd.dma_start(
                        x[:], dr_in["q"].ap()[b, c * P:(c + 1) * P, :])
                    for h in range(2):
                        for s2 in range(2):
                            nc.tensor.matmul(
                                halves[h][:, s2 * 512:(s2 + 1) * 512],
                                lhsT=w_sb["q"][:, c, :],
                                rhs=x[:, h * 1024 + s2 * 512:
                                      h * 1024 + (s2 + 1) * 512],
                                start=(c == 0), stop=(c == DC - 1))
                qt = proj.tile([P, S], BF16, tag="qT", name="qT")
                for h in range(2):
                    nc.vector.tensor_scalar_add(
                        qt[:, h * 1024:(h + 1) * 1024],
                        halves[h][:], b_sb["q"][:])
                return qt

            def emit_vt_chunk(b, vt_ps, c):
                """One D-chunk of the V projection into vt_ps ("acc")."""
                x = stream.tile([P, S], BF16, tag="stream", name="x")
                nc.gpsimd.dma_start(
                    x[:], dr_in["v"].ap()[b, c * P:(c + 1) * P, :])
                for r in range(4):
                    nc.tensor.matmul(
                        vt_ps[:, r * 512:(r + 1) * 512],
                        lhsT=w_sb["v"][:, c, :],
                        rhs=x[:, r * 512:(r + 1) * 512],
                        start=(c == 0), stop=(c == DC - 1))

            def emit_v_finish(vt_ps):
                """Bias-add, PE-transpose VT -> V [S, d]."""
                vt_sb = proj.tile([P, S], BF16, tag="vT", name="vT")
                for h in range(2):
                    nc.vector.tensor_scalar_add(
                        vt_sb[:, h * 1024:(h + 1) * 1024],
                        vt_ps[:, h * 1024:(h + 1) * 1024], b_sb["v"][:])
                v_sb = proj.tile([P, KC, P], BF16, tag="v", name="v")
                trt = ps_acc.tile([P, S], BF16, tag="acc", name="tr_ps")
                for idx in range(KC):
                    nc.tensor.transpose(
                        trt[:, idx * P:(idx + 1) * P],
                        vt_sb[:, idx * P:(idx + 1) * P], ident[:])
                for g in range(2):
                    nc.vector.tensor_copy(
                        v_sb[:, g * 8:(g + 1) * 8, :],
                        trt[:, g * 1024:(g + 1) * 1024].rearrange(
                            "p (a e) -> p a e", a=8))
                return v_sb

            def emit_kchunk(b, sl):
                """K super-chunk: [D, 256] slab -> kct [d, 256] bf16
                (2 k-chunks worth of KT), so scores start on the first
                slab instead of after the whole K projection."""
                xk = stream.tile([P, DC, 256], BF16, tag="stream",
                                 name="xk")
                nc.gpsimd.dma_start(
                    xk[:],
                    dr_in["k"].ap()[b, :, sl * 256:(sl + 1) * 256]
                    .rearrange("(c p) s -> p c s", p=P))
                kps = ps_big.tile([P, 256], F32, tag="big", name="k_ps")
                for c in range(DC):
                    nc.tensor.matmul(
                        kps[:], lhsT=w_sb["k"][:, c, :], rhs=xk[:, c, :],
                        start=(c == 0), stop=(c == DC - 1))
                kct = kctp.tile([P, 256], BF16, tag="kt", name="kct")
                nc.vector.tensor_scalar_add(kct[:], kps[:], b_sb["k"][:])
                return kct

            def emit_scores(qt, lhsT_ap):
                """One k-chunk of scores^T + exp + 1/Z."""
                pt = ptp.tile([P, S], BF16, tag="pt", name="pt")
                zz = stats.tile([P, 2], F32, tag="z", name="zz")
                for h in range(2):
                    sc = ps_big.tile([P, 1024], F32, tag="big",
                                     name="sc_ps")
                    for s2 in range(2):
                        nc.tensor.matmul(
                            sc[:, s2 * 512:(s2 + 1) * 512],
                            lhsT=lhsT_ap,
                            rhs=qt[:, h * 1024 + s2 * 512:
                                   h * 1024 + (s2 + 1) * 512],
                            start=True, stop=True)
                    nc.scalar.activation(
                        pt[:, h * 1024:(h + 1) * 1024], sc[:],
                        func=mybir.ActivationFunctionType.Exp,
                        scale=SCALE, accum_out=zz[:, h:h + 1])
                rec = recp.tile([P, 1], F32, tag="rec", name="rec")
                nc.vector.tensor_reduce(
                    rec[:], zz[:], axis=mybir.AxisListType.X,
                    op=mybir.AluOpType.add)
                nc.vector.reciprocal(rec[:], rec[:])
                return pt, rec

            def emit_ht_mm(ht, v_sb, pt, rec, kc):
                vs = stats.tile([P, P], BF16, tag="vs", name="vs")
                nc.vector.tensor_scalar_mul(vs[:], v_sb[:, kc, :], rec[:])
                for st in range(4):
                    nc.tensor.matmul(
                        ht[:, st * 512:(st + 1) * 512],
                        lhsT=vs[:],
                        rhs=pt[:, st * 512:(st + 1) * 512],
                        start=(kc == 0), stop=(kc == KC - 1))

            def emit_out(b, ht):
                out_sb = osb.tile([P, S], F32, tag="osb", name="out_sb")
                nc.vector.tensor_copy(out_sb[:], ht[:])
                nc.sync.dma_start(dr_out.ap()[b], out_sb[:])

            for b in range(B_LOC):
                qt = emit_qt(b)
                # whole-K projection
                khalves = [ps_big.tile([P, 1024], F32, tag="big",
                                       name="k_ps") for _ in range(2)]
                for c in range(DC):
                    x = stream.tile([P, S], BF16, tag="stream", name="x")
                    nc.gpsimd.dma_start(
                        x[:], dr_in["k"].ap()[b, c * P:(c + 1) * P, :])
                    for h in range(2):
                        for s2 in range(2):
                            nc.tensor.matmul(
                                khalves[h][:, s2 * 512:(s2 + 1) * 512],
                                lhsT=w_sb["k"][:, c, :],
                                rhs=x[:, h * 1024 + s2 * 512:
                                      h * 1024 + (s2 + 1) * 512],
                                start=(c == 0), stop=(c == DC - 1))
                kt = proj.tile([P, S], BF16, tag="kT", name="kT")
                for h in range(2):
                    nc.vector.tensor_scalar_add(
                        kt[:, h * 1024:(h + 1) * 1024],
                        khalves[h][:], b_sb["k"][:])

                # scores/exp chain with the V projection interleaved at
                # odd kc (V matmuls live in the "acc" PSUM region so the
                # sc double-buffer is undisturbed; each v-chunk's stream
                # slot frees as its matmuls retire)
                vt_ps = ps_acc.tile([P, S], F32, tag="acc", name="vt_ps")
                pts = []
                recs = []
                for kc in range(KC):
                    pt, rec = emit_scores(qt, kt[:, kc * P:(kc + 1) * P])
                    pts.append(pt)
                    recs.append(rec)
                    if kc % 2 == 1:
                        emit_vt_chunk(b, vt_ps, kc // 2)
                v_sb = emit_v_finish(vt_ps)
                ht = ps_acc.tile([P, S], F32, tag="acc", name="ht")
                for kc in range(KC):
                    emit_ht_mm(ht, v_sb, pts[kc], recs[kc], kc)
                emit_out(b, ht)

    nc.compile()
    return nc


def _get_nc():
    global _BUILT
    if _BUILT is None:
        _BUILT = build()
    return _BUILT


def kernel(inp_q, inp_k, inp_v, Wq_kernel, Wq_bias, Wk_kernel, Wk_bias,
           Wv_kernel, Wv_bias):
    from concourse.bass_utils import run_bass_kernel_spmd

    nc = _get_nc()

    inp = {"q": np.asarray(inp_q, dtype=np.float32),
           "k": np.asarray(inp_k, dtype=np.float32),
           "v": np.asarray(inp_v, dtype=np.float32)}
    w = {"q": np.ascontiguousarray(np.asarray(Wq_kernel, dtype=np.float32)),
         "k": np.ascontiguousarray(np.asarray(Wk_kernel, dtype=np.float32)),
         "v": np.ascontiguousarray(np.asarray(Wv_kernel, dtype=np.float32))}
    bias = {"q": np.ascontiguousarray(np.asarray(Wq_bias, dtype=np.float32)),
            "k": np.ascontiguousarray(np.asarray(Wk_bias, dtype=np.float32)),
            "v": np.ascontiguousarray(np.asarray(Wv_bias, dtype=np.float32))}

    in_maps = []
    for c in range(N_CORES):
        m = {}
        for t in ("q", "k", "v"):
            # [2, S, D] -> [2, D, S] contiguous (pure layout marshalling)
            m[f"{t}T"] = np.ascontiguousarray(
                inp[t][c * B_LOC:(c + 1) * B_LOC].transpose(0, 2, 1))
            m[f"w{t}"] = w[t]
            m[f"b{t}"] = bias[t]
        in_maps.append(m)

    res = run_bass_kernel_spmd(nc, in_maps, list(range(N_CORES)))

    out = np.empty((N_CORES * B_LOC, S, P), dtype=np.float32)
    for c in range(N_CORES):
        # [2, P, S] -> [2, S, P]
        out[c * B_LOC:(c + 1) * B_LOC] = (
            res.results[c]["out"].transpose(0, 2, 1))
    return out


# revision 23
# speedup vs baseline: 1.1104x; 1.0369x over previous
"""Trainium2 Bass kernel for the AttentionLayer problem.

Math (per batch):
    Q = inp_q @ Wq + bq            [S, d]
    K = inp_k @ Wk + bk            [S, d]
    V = inp_v @ Wv + bv            [S, d]
    sc = Q @ K^T / sqrt(d)         [Sq, Sk]
    S_ = softmax(sc, axis=0)       (over the QUERY axis)
    H = S_ @ V                     [Sq, d]

Device-side layout strategy (per core, 2 batches):
  * Host feeds transposed activations xT = x^T [D, S] so every matmul
    contracts over the SBUF partition dim with zero on-chip transposes
    of the big activations.
  * Projections produce QT/KT/VT in [d, S] layout (d = 128 partitions).
  * scores^T [k, q] = (KT-slice)^T @ QT, so softmax-over-q is a
    free-axis row reduction: one ACT pass does exp(scale*x) and the
    row sum Z[k].  No max-subtraction is needed: |sc/sqrt(d)| <~ 6 for
    randn inputs, exp() is exact in f32 there.
  * Normalization is folded into V: vs[k, :] = V[k, :] / Z[k], then
    H^T [d, q] += vs-slice^T @ P^T accumulates over k-chunks in PSUM.
  * Host un-transposes H^T -> H.
Compute dtype bf16 (f32 PSUM accumulate), stats in f32.
"""

import math
import sys

sys.path.insert(0, "/opt/trn_rl_repo")

import numpy as np

import concourse.bass as bass  # noqa: E402
import concourse.tile as tile  # noqa: E402
from concourse import bacc, mybir  # noqa: E402
from concourse.masks import make_identity  # noqa: E402

P = 128          # partitions / head dim d
S = 2048         # sequence length
D = 1024         # model dim
DC = D // P      # D chunks (8)
KC = S // P      # key chunks (16)
B_LOC = 2        # batches per core
N_CORES = 8
SCALE = 1.0 / math.sqrt(P)

F32 = mybir.dt.float32
BF16 = mybir.dt.bfloat16

_BUILT = None  # cached (nc,) so repeated kernel() calls reuse the NEFF


def build():
    nc = bacc.Bacc("TRN2", target_bir_lowering=False, debug=False,
                   num_devices=N_CORES)

    dr_in = {}
    for t in ("q", "k", "v"):
        dr_in[t] = nc.dram_tensor(f"{t}T", [B_LOC, D, S], F32,
                                  kind="ExternalInput")
    dr_w = {t: nc.dram_tensor(f"w{t}", [D, P], F32, kind="ExternalInput")
            for t in ("q", "k", "v")}
    dr_b = {t: nc.dram_tensor(f"b{t}", [P], F32, kind="ExternalInput")
            for t in ("q", "k", "v")}
    dr_out = nc.dram_tensor("out", [B_LOC, P, S], F32, kind="ExternalOutput")

    with tile.TileContext(nc) as tc:
        with (
            tc.tile_pool(name="const", bufs=1) as const,
            tc.tile_pool(name="stream", bufs=6) as stream,
            tc.tile_pool(name="proj", bufs=2) as proj,
            tc.tile_pool(name="kctp", bufs=10) as kctp,
            tc.tile_pool(name="ptp", bufs=14) as ptp,
            tc.tile_pool(name="stats", bufs=4) as stats,
            tc.tile_pool(name="recp", bufs=18) as recp,
            tc.tile_pool(name="osb", bufs=2) as osb,
            tc.tile_pool(name="ps_big", bufs=2, space="PSUM") as ps_big,
            tc.tile_pool(name="ps_acc", bufs=1, space="PSUM") as ps_acc,
        ):
            # ---- constants ----
            w_sb = {}
            b_sb = {}
            for t in ("q", "k", "v"):
                w_sb[t] = const.tile([P, DC, P], BF16, tag=f"w{t}", name=f"w{t}")
                # [D, P] -> [p, c, e]; cast f32->bf16 inline (SWDGE)
                nc.gpsimd.dma_start(
                    w_sb[t][:],
                    dr_w[t].ap().rearrange("(c p) e -> p c e", p=P))
                b_sb[t] = const.tile([P, 1], F32, tag=f"b{t}", name=f"b{t}")
                nc.sync.dma_start(
                    b_sb[t][:],
                    dr_b[t].ap().rearrange("(p o) -> p o", o=1))
            ident = const.tile([P, P], BF16, tag="ident", name="ident")
            make_identity(nc, ident[:])

            def proj_dbl_chunk(t, b, cc, sinks):
                """Load a 2MB double D-chunk (two 128-row slabs in one
                dma_start for better DMA efficiency) and run its
                projection matmuls.  sinks(c, rhs_slice_fn) emits them."""
                x = stream.tile([P, 2, S], BF16, tag="stream", name="x")
                nc.gpsimd.dma_start(
                    x[:],
                    dr_in[t].ap()[b, cc * 2 * P:(cc + 1) * 2 * P, :]
                    .rearrange("(two p) s -> p two s", two=2))
                for two in range(2):
                    sinks(cc * 2 + two, x[:, two, :])

            def emit_qt(b, t="q", tag="qT"):
                """Q/K projection: 4 double-chunks -> [d, S] bf16."""
                halves = [ps_big.tile([P, 1024], F32, tag="big",
                                      name="q_ps") for _ in range(2)]

                def sinks(c, rhs):
                    for h in range(2):
                        for s2 in range(2):
                            nc.tensor.matmul(
                                halves[h][:, s2 * 512:(s2 + 1) * 512],
                                lhsT=w_sb[t][:, c, :],
                                rhs=rhs[:, h * 1024 + s2 * 512:
                                        h * 1024 + (s2 + 1) * 512],
                                start=(c == 0), stop=(c == DC - 1))

                for cc in range(DC // 2):
                    proj_dbl_chunk(t, b, cc, sinks)
                out = proj.tile([P, S], BF16, tag=tag, name=tag)
                for h in range(2):
                    nc.vector.tensor_scalar_add(
                        out[:, h * 1024:(h + 1) * 1024],
                        halves[h][:], b_sb[t][:])
                return out

            def emit_vt_chunk(b, vt_ps, cc):
                """One double D-chunk of the V projection ("acc")."""

                def sinks(c, rhs):
                    for r in range(4):
                        nc.tensor.matmul(
                            vt_ps[:, r * 512:(r + 1) * 512],
                            lhsT=w_sb["v"][:, c, :],
                            rhs=rhs[:, r * 512:(r + 1) * 512],
                            start=(c == 0), stop=(c == DC - 1))

                proj_dbl_chunk("v", b, cc, sinks)

            def emit_v_finish(vt_ps):
                """Bias-add, PE-transpose VT -> V [S, d]."""
                vt_sb = proj.tile([P, S], BF16, tag="vT", name="vT")
                for h in range(2):
                    nc.vector.tensor_scalar_add(
                        vt_sb[:, h * 1024:(h + 1) * 1024],
                        vt_ps[:, h * 1024:(h + 1) * 1024], b_sb["v"][:])
                v_sb = proj.tile([P, KC, P], BF16, tag="v", name="v")
                trt = ps_acc.tile([P, S], BF16, tag="acc", name="tr_ps")
                for idx in range(KC):
                    nc.tensor.transpose(
                        trt[:, idx * P:(idx + 1) * P],
                        vt_sb[:, idx * P:(idx + 1) * P], ident[:])
                for g in range(2):
                    nc.vector.tensor_copy(
                        v_sb[:, g * 8:(g + 1) * 8, :],
                        trt[:, g * 1024:(g + 1) * 1024].rearrange(
                            "p (a e) -> p a e", a=8))
                return v_sb

            def emit_kchunk(b, sl):
                """K super-chunk: [D, 256] slab -> kct [d, 256] bf16
                (2 k-chunks worth of KT), so scores start on the first
                slab instead of after the whole K projection."""
                xk = stream.tile([P, DC, 256], BF16, tag="stream",
                                 name="xk")
                nc.gpsimd.dma_start(
                    xk[:],
                    dr_in["k"].ap()[b, :, sl * 256:(sl + 1) * 256]
                    .rearrange("(c p) s -> p c s", p=P))
                kps = ps_big.tile([P, 256], F32, tag="big", name="k_ps")
                for c in range(DC):
                    nc.tensor.matmul(
                        kps[:], lhsT=w_sb["k"][:, c, :], rhs=xk[:, c, :],
                        start=(c == 0), stop=(c == DC - 1))
                kct = kctp.tile([P, 256], BF16, tag="kt", name="kct")
                nc.vector.tensor_scalar_add(kct[:], kps[:], b_sb["k"][:])
                return kct

            def emit_scores(qt, lhsT_ap):
                """One k-chunk of scores^T + exp + 1/Z."""
                pt = ptp.tile([P, S], BF16, tag="pt", name="pt")
                zz = stats.tile([P, 2], F32, tag="z", name="zz")
                for h in range(2):
                    sc = ps_big.tile([P, 1024], F32, tag="big",
                                     name="sc_ps")
                    for s2 in range(2):
                        nc.tensor.matmul(
                            sc[:, s2 * 512:(s2 + 1) * 512],
                            lhsT=lhsT_ap,
                            rhs=qt[:, h * 1024 + s2 * 512:
                                   h * 1024 + (s2 + 1) * 512],
                            start=True, stop=True)
                    nc.scalar.activation(
                        pt[:, h * 1024:(h + 1) * 1024], sc[:],
                        func=mybir.ActivationFunctionType.Exp,
                        scale=SCALE, accum_out=zz[:, h:h + 1])
                rec = recp.tile([P, 1], F32, tag="rec", name="rec")
                nc.vector.tensor_reduce(
                    rec[:], zz[:], axis=mybir.AxisListType.X,
                    op=mybir.AluOpType.add)
                nc.vector.reciprocal(rec[:], rec[:])
                return pt, rec

            def emit_ht_mm(ht, v_sb, pt, rec, kc):
                vs = stats.tile([P, P], BF16, tag="vs", name="vs")
                nc.vector.tensor_scalar_mul(vs[:], v_sb[:, kc, :], rec[:])
                for st in range(4):
                    nc.tensor.matmul(
                        ht[:, st * 512:(st + 1) * 512],
                        lhsT=vs[:],
                        rhs=pt[:, st * 512:(st + 1) * 512],
                        start=(kc == 0), stop=(kc == KC - 1))

            def emit_out(b, ht):
                out_sb = osb.tile([P, S], F32, tag="osb", name="out_sb")
                nc.vector.tensor_copy(out_sb[:], ht[:])
                nc.sync.dma_start(dr_out.ap()[b], out_sb[:])

            for b in range(B_LOC):
                qt = emit_qt(b, "q", "qT")
                kt = emit_qt(b, "k", "kT")

                # scores/exp chain with the V projection interleaved
                # every 4th kc (V matmuls live in the "acc" PSUM region
                # so the sc double-buffer is undisturbed; each v-chunk's
                # stream slot frees as its matmuls retire)
                vt_ps = ps_acc.tile([P, S], F32, tag="acc", name="vt_ps")
                pts = []
                recs = []
                for kc in range(KC):
                    pt, rec = emit_scores(qt, kt[:, kc * P:(kc + 1) * P])
                    pts.append(pt)
                    recs.append(rec)
                    if kc % 4 == 1:
                        emit_vt_chunk(b, vt_ps, kc // 4)
                v_sb = emit_v_finish(vt_ps)
                ht = ps_acc.tile([P, S], F32, tag="acc", name="ht")
                for kc in range(KC):
                    emit_ht_mm(ht, v_sb, pts[kc], recs[kc], kc)
                emit_out(b, ht)

    nc.compile()
    return nc


def _get_nc():
    global _BUILT
    if _BUILT is None:
        _BUILT = build()
    return _BUILT


def kernel(inp_q, inp_k, inp_v, Wq_kernel, Wq_bias, Wk_kernel, Wk_bias,
           Wv_kernel, Wv_bias):
    from concourse.bass_utils import run_bass_kernel_spmd

    nc = _get_nc()

    inp = {"q": np.asarray(inp_q, dtype=np.float32),
           "k": np.asarray(inp_k, dtype=np.float32),
           "v": np.asarray(inp_v, dtype=np.float32)}
    w = {"q": np.ascontiguousarray(np.asarray(Wq_kernel, dtype=np.float32)),
         "k": np.ascontiguousarray(np.asarray(Wk_kernel, dtype=np.float32)),
         "v": np.ascontiguousarray(np.asarray(Wv_kernel, dtype=np.float32))}
    bias = {"q": np.ascontiguousarray(np.asarray(Wq_bias, dtype=np.float32)),
            "k": np.ascontiguousarray(np.asarray(Wk_bias, dtype=np.float32)),
            "v": np.ascontiguousarray(np.asarray(Wv_bias, dtype=np.float32))}

    in_maps = []
    for c in range(N_CORES):
        m = {}
        for t in ("q", "k", "v"):
            # [2, S, D] -> [2, D, S] contiguous (pure layout marshalling)
            m[f"{t}T"] = np.ascontiguousarray(
                inp[t][c * B_LOC:(c + 1) * B_LOC].transpose(0, 2, 1))
            m[f"w{t}"] = w[t]
            m[f"b{t}"] = bias[t]
        in_maps.append(m)

    res = run_bass_kernel_spmd(nc, in_maps, list(range(N_CORES)))

    out = np.empty((N_CORES * B_LOC, S, P), dtype=np.float32)
    for c in range(N_CORES):
        # [2, P, S] -> [2, S, P]
        out[c * B_LOC:(c + 1) * B_LOC] = (
            res.results[c]["out"].transpose(0, 2, 1))
    return out


# revision 24
# speedup vs baseline: 1.1631x; 1.0474x over previous
"""Trainium2 Bass kernel for the AttentionLayer problem.

Math (per batch):
    Q = inp_q @ Wq + bq            [S, d]
    K = inp_k @ Wk + bk            [S, d]
    V = inp_v @ Wv + bv            [S, d]
    sc = Q @ K^T / sqrt(d)         [Sq, Sk]
    S_ = softmax(sc, axis=0)       (over the QUERY axis)
    H = S_ @ V                     [Sq, d]

Device-side layout strategy (per core, 2 batches):
  * Host feeds transposed activations xT = x^T [D, S] so every matmul
    contracts over the SBUF partition dim with zero on-chip transposes
    of the big activations.
  * Projections produce QT/KT/VT in [d, S] layout (d = 128 partitions).
  * scores^T [k, q] = (KT-slice)^T @ QT, so softmax-over-q is a
    free-axis row reduction: one ACT pass does exp(scale*x) and the
    row sum Z[k].  No max-subtraction is needed: |sc/sqrt(d)| <~ 6 for
    randn inputs, exp() is exact in f32 there.
  * Normalization is folded into V: vs[k, :] = V[k, :] / Z[k], then
    H^T [d, q] += vs-slice^T @ P^T accumulates over k-chunks in PSUM.
  * Host un-transposes H^T -> H.
Compute dtype bf16 (f32 PSUM accumulate), stats in f32.
"""

import math
import sys

sys.path.insert(0, "/opt/trn_rl_repo")

import numpy as np

import concourse.bass as bass  # noqa: E402
import concourse.tile as tile  # noqa: E402
from concourse import bacc, mybir  # noqa: E402
from concourse.masks import make_identity  # noqa: E402

P = 128          # partitions / head dim d
S = 2048         # sequence length
D = 1024         # model dim
DC = D // P      # D chunks (8)
KC = S // P      # key chunks (16)
B_LOC = 2        # batches per core
N_CORES = 8
SCALE = 1.0 / math.sqrt(P)

F32 = mybir.dt.float32
BF16 = mybir.dt.bfloat16

_BUILT = None  # cached (nc,) so repeated kernel() calls reuse the NEFF


def build():
    nc = bacc.Bacc("TRN2", target_bir_lowering=False, debug=False,
                   num_devices=N_CORES)

    dr_in = {}
    for t in ("q", "k", "v"):
        dr_in[t] = nc.dram_tensor(f"{t}T", [B_LOC, D, S], F32,
                                  kind="ExternalInput")
    dr_w = {t: nc.dram_tensor(f"w{t}", [D, P], F32, kind="ExternalInput")
            for t in ("q", "k", "v")}
    dr_b = {t: nc.dram_tensor(f"b{t}", [P], F32, kind="ExternalInput")
            for t in ("q", "k", "v")}
    dr_out = nc.dram_tensor("out", [B_LOC, P, S], F32, kind="ExternalOutput")

    with tile.TileContext(nc) as tc:
        with (
            tc.tile_pool(name="const", bufs=1) as const,
            tc.tile_pool(name="stream", bufs=6) as stream,
            tc.tile_pool(name="proj", bufs=2) as proj,
            tc.tile_pool(name="kctp", bufs=10) as kctp,
            tc.tile_pool(name="ptp", bufs=14) as ptp,
            tc.tile_pool(name="stats", bufs=4) as stats,
            tc.tile_pool(name="recp", bufs=18) as recp,
            tc.tile_pool(name="osb", bufs=2) as osb,
            tc.tile_pool(name="ps_big", bufs=2, space="PSUM") as ps_big,
            tc.tile_pool(name="ps_acc", bufs=1, space="PSUM") as ps_acc,
        ):
            # ---- constants ----
            w_sb = {}
            b_sb = {}
            for t in ("q", "k", "v"):
                w_sb[t] = const.tile([P, DC, P], BF16, tag=f"w{t}", name=f"w{t}")
                # [D, P] -> [p, c, e]; cast f32->bf16 inline (SWDGE)
                nc.gpsimd.dma_start(
                    w_sb[t][:],
                    dr_w[t].ap().rearrange("(c p) e -> p c e", p=P))
                b_sb[t] = const.tile([P, 1], F32, tag=f"b{t}", name=f"b{t}")
                nc.sync.dma_start(
                    b_sb[t][:],
                    dr_b[t].ap().rearrange("(p o) -> p o", o=1))
            ident = const.tile([P, P], BF16, tag="ident", name="ident")
            make_identity(nc, ident[:])

            def proj_dbl_chunk(t, b, cc, sinks):
                """Load a 2MB double D-chunk (two 128-row slabs in one
                dma_start for better DMA efficiency) and run its
                projection matmuls.  sinks(c, rhs_slice_fn) emits them."""
                x = stream.tile([P, 2, S], BF16, tag="stream", name="x")
                nc.gpsimd.dma_start(
                    x[:],
                    dr_in[t].ap()[b, cc * 2 * P:(cc + 1) * 2 * P, :]
                    .rearrange("(two p) s -> p two s", two=2))
                for two in range(2):
                    sinks(cc * 2 + two, x[:, two, :])

            def emit_qt(b, t="q", tag="qT"):
                """Q/K projection: 4 double-chunks -> [d, S] bf16."""
                halves = [ps_big.tile([P, 1024], F32, tag="big",
                                      name="q_ps") for _ in range(2)]

                def sinks(c, rhs):
                    for h in range(2):
                        for s2 in range(2):
                            nc.tensor.matmul(
                                halves[h][:, s2 * 512:(s2 + 1) * 512],
                                lhsT=w_sb[t][:, c, :],
                                rhs=rhs[:, h * 1024 + s2 * 512:
                                        h * 1024 + (s2 + 1) * 512],
                                start=(c == 0), stop=(c == DC - 1))

                for cc in range(DC // 2):
                    proj_dbl_chunk(t, b, cc, sinks)
                out = proj.tile([P, S], BF16, tag=tag, name=tag)
                for h in range(2):
                    nc.vector.tensor_scalar_add(
                        out[:, h * 1024:(h + 1) * 1024],
                        halves[h][:], b_sb[t][:])
                return out

            def emit_vt_chunk(b, vt_ps, cc):
                """One double D-chunk of the V projection ("acc")."""

                def sinks(c, rhs):
                    for r in range(4):
                        nc.tensor.matmul(
                            vt_ps[:, r * 512:(r + 1) * 512],
                            lhsT=w_sb["v"][:, c, :],
                            rhs=rhs[:, r * 512:(r + 1) * 512],
                            start=(c == 0), stop=(c == DC - 1))

                proj_dbl_chunk("v", b, cc, sinks)

            def emit_v_finish(vt_ps):
                """Bias-add, PE-transpose VT -> V [S, d]."""
                vt_sb = proj.tile([P, S], BF16, tag="vT", name="vT")
                for h in range(2):
                    nc.vector.tensor_scalar_add(
                        vt_sb[:, h * 1024:(h + 1) * 1024],
                        vt_ps[:, h * 1024:(h + 1) * 1024], b_sb["v"][:])
                v_sb = proj.tile([P, KC, P], BF16, tag="v", name="v")
                trt = ps_acc.tile([P, S], BF16, tag="acc", name="tr_ps")
                for idx in range(KC):
                    nc.tensor.transpose(
                        trt[:, idx * P:(idx + 1) * P],
                        vt_sb[:, idx * P:(idx + 1) * P], ident[:])
                for g in range(2):
                    nc.vector.tensor_copy(
                        v_sb[:, g * 8:(g + 1) * 8, :],
                        trt[:, g * 1024:(g + 1) * 1024].rearrange(
                            "p (a e) -> p a e", a=8))
                return v_sb

            def emit_kchunk(b, sl):
                """K super-chunk: [D, 256] slab -> kct [d, 256] bf16
                (2 k-chunks worth of KT), so scores start on the first
                slab instead of after the whole K projection."""
                xk = stream.tile([P, DC, 256], BF16, tag="stream",
                                 name="xk")
                nc.gpsimd.dma_start(
                    xk[:],
                    dr_in["k"].ap()[b, :, sl * 256:(sl + 1) * 256]
                    .rearrange("(c p) s -> p c s", p=P))
                kps = ps_big.tile([P, 256], F32, tag="big", name="k_ps")
                for c in range(DC):
                    nc.tensor.matmul(
                        kps[:], lhsT=w_sb["k"][:, c, :], rhs=xk[:, c, :],
                        start=(c == 0), stop=(c == DC - 1))
                kct = kctp.tile([P, 256], BF16, tag="kt", name="kct")
                nc.vector.tensor_scalar_add(kct[:], kps[:], b_sb["k"][:])
                return kct

            def emit_scores(qt, lhsT_ap):
                """One k-chunk of scores^T + exp + 1/Z."""
                pt = ptp.tile([P, S], BF16, tag="pt", name="pt")
                zz = stats.tile([P, 2], F32, tag="z", name="zz")
                for h in range(2):
                    sc = ps_big.tile([P, 1024], F32, tag="big",
                                     name="sc_ps")
                    for s2 in range(2):
                        nc.tensor.matmul(
                            sc[:, s2 * 512:(s2 + 1) * 512],
                            lhsT=lhsT_ap,
                            rhs=qt[:, h * 1024 + s2 * 512:
                                   h * 1024 + (s2 + 1) * 512],
                            start=True, stop=True)
                    nc.scalar.activation(
                        pt[:, h * 1024:(h + 1) * 1024], sc[:],
                        func=mybir.ActivationFunctionType.Exp,
                        scale=SCALE, accum_out=zz[:, h:h + 1])
                rec = recp.tile([P, 1], F32, tag="rec", name="rec")
                nc.vector.tensor_reduce(
                    rec[:], zz[:], axis=mybir.AxisListType.X,
                    op=mybir.AluOpType.add)
                nc.vector.reciprocal(rec[:], rec[:])
                return pt, rec

            def emit_ht_mm(ht, v_sb, pt, rec, kc):
                vs = stats.tile([P, P], BF16, tag="vs", name="vs")
                nc.vector.tensor_scalar_mul(vs[:], v_sb[:, kc, :], rec[:])
                for st in range(4):
                    nc.tensor.matmul(
                        ht[:, st * 512:(st + 1) * 512],
                        lhsT=vs[:],
                        rhs=pt[:, st * 512:(st + 1) * 512],
                        start=(kc == 0), stop=(kc == KC - 1))

            def emit_out(b, ht):
                out_sb = osb.tile([P, S], F32, tag="osb", name="out_sb")
                nc.vector.tensor_copy(out_sb[:], ht[:])
                nc.sync.dma_start(dr_out.ap()[b], out_sb[:])

            for b in range(B_LOC):
                qt = emit_qt(b, "q", "qT")
                kt = emit_qt(b, "k", "kT")

                # scores/exp chain with the V projection interleaved
                # every 4th kc (V matmuls live in the "acc" PSUM region
                # so the sc double-buffer is undisturbed; each v-chunk's
                # stream slot frees as its matmuls retire)
                vt_ps = ps_acc.tile([P, S], F32, tag="acc", name="vt_ps")
                pts = []
                recs = []
                for kc in range(KC):
                    pt, rec = emit_scores(qt, kt[:, kc * P:(kc + 1) * P])
                    pts.append(pt)
                    recs.append(rec)
                    if kc in (1, 4, 7, 10):
                        emit_vt_chunk(b, vt_ps, (kc - 1) // 3)
                v_sb = emit_v_finish(vt_ps)
                ht = ps_acc.tile([P, S], F32, tag="acc", name="ht")
                for kc in range(KC):
                    emit_ht_mm(ht, v_sb, pts[kc], recs[kc], kc)
                emit_out(b, ht)

    nc.compile()
    return nc


def _get_nc():
    global _BUILT
    if _BUILT is None:
        _BUILT = build()
    return _BUILT


def kernel(inp_q, inp_k, inp_v, Wq_kernel, Wq_bias, Wk_kernel, Wk_bias,
           Wv_kernel, Wv_bias):
    from concourse.bass_utils import run_bass_kernel_spmd

    nc = _get_nc()

    inp = {"q": np.asarray(inp_q, dtype=np.float32),
           "k": np.asarray(inp_k, dtype=np.float32),
           "v": np.asarray(inp_v, dtype=np.float32)}
    w = {"q": np.ascontiguousarray(np.asarray(Wq_kernel, dtype=np.float32)),
         "k": np.ascontiguousarray(np.asarray(Wk_kernel, dtype=np.float32)),
         "v": np.ascontiguousarray(np.asarray(Wv_kernel, dtype=np.float32))}
    bias = {"q": np.ascontiguousarray(np.asarray(Wq_bias, dtype=np.float32)),
            "k": np.ascontiguousarray(np.asarray(Wk_bias, dtype=np.float32)),
            "v": np.ascontiguousarray(np.asarray(Wv_bias, dtype=np.float32))}

    in_maps = []
    for c in range(N_CORES):
        m = {}
        for t in ("q", "k", "v"):
            # [2, S, D] -> [2, D, S] contiguous (pure layout marshalling)
            m[f"{t}T"] = np.ascontiguousarray(
                inp[t][c * B_LOC:(c + 1) * B_LOC].transpose(0, 2, 1))
            m[f"w{t}"] = w[t]
            m[f"b{t}"] = bias[t]
        in_maps.append(m)

    res = run_bass_kernel_spmd(nc, in_maps, list(range(N_CORES)))

    out = np.empty((N_CORES * B_LOC, S, P), dtype=np.float32)
    for c in range(N_CORES):
        # [2, P, S] -> [2, S, P]
        out[c * B_LOC:(c + 1) * B_LOC] = (
            res.results[c]["out"].transpose(0, 2, 1))
    return out


# revision 30
# speedup vs baseline: 1.2275x; 1.0554x over previous
"""Trainium2 Bass kernel for the AttentionLayer problem.

Math (per batch):
    Q = inp_q @ Wq + bq            [S, d]
    K = inp_k @ Wk + bk            [S, d]
    V = inp_v @ Wv + bv            [S, d]
    sc = Q @ K^T / sqrt(d)         [Sq, Sk]
    S_ = softmax(sc, axis=0)       (over the QUERY axis)
    H = S_ @ V                     [Sq, d]

Device-side layout strategy (per core, 2 batches):
  * Host feeds transposed activations xT = x^T [D, S] so every matmul
    contracts over the SBUF partition dim with zero on-chip transposes
    of the big activations.
  * Projections produce QT/KT/VT in [d, S] layout (d = 128 partitions).
  * scores^T [k, q] = (KT-slice)^T @ QT, so softmax-over-q is a
    free-axis row reduction: one ACT pass does exp(scale*x) and the
    row sum Z[k].  No max-subtraction is needed: |sc/sqrt(d)| <~ 6 for
    randn inputs, exp() is exact in f32 there.
  * Normalization is folded into V: vs[k, :] = V[k, :] / Z[k], then
    H^T [d, q] += vs-slice^T @ P^T accumulates over k-chunks in PSUM.
  * Host un-transposes H^T -> H.
Compute dtype bf16 (f32 PSUM accumulate), stats in f32.
"""

import math
import sys

sys.path.insert(0, "/opt/trn_rl_repo")

import numpy as np

import concourse.bass as bass  # noqa: E402
import concourse.tile as tile  # noqa: E402
from concourse import bacc, mybir  # noqa: E402
from concourse.masks import make_identity  # noqa: E402

P = 128          # partitions / head dim d
S = 2048         # sequence length
D = 1024         # model dim
DC = D // P      # D chunks (8)
KC = S // P      # key chunks (16)
B_LOC = 2        # batches per core
N_CORES = 8
SCALE = 1.0 / math.sqrt(P)

F32 = mybir.dt.float32
BF16 = mybir.dt.bfloat16

_BUILT = None  # cached (nc,) so repeated kernel() calls reuse the NEFF


def build():
    nc = bacc.Bacc("TRN2", target_bir_lowering=False, debug=False,
                   num_devices=N_CORES)

    dr_in = {}
    for t in ("q", "k", "v"):
        dr_in[t] = nc.dram_tensor(f"{t}T", [B_LOC, D, S], F32,
                                  kind="ExternalInput")
    dr_w = {t: nc.dram_tensor(f"w{t}", [D, P], F32, kind="ExternalInput")
            for t in ("q", "k", "v")}
    dr_b = {t: nc.dram_tensor(f"b{t}", [P], F32, kind="ExternalInput")
            for t in ("q", "k", "v")}
    dr_out = nc.dram_tensor("out", [B_LOC, P, S], F32, kind="ExternalOutput")

    with tile.TileContext(nc) as tc:
        with (
            tc.tile_pool(name="const", bufs=1) as const,
            tc.tile_pool(name="stream", bufs=8) as stream,
            tc.tile_pool(name="proj", bufs=2) as proj,
            tc.tile_pool(name="kctp", bufs=10) as kctp,
            tc.tile_pool(name="ptp", bufs=16) as ptp,
            tc.tile_pool(name="stats", bufs=4) as stats,
            tc.tile_pool(name="recp", bufs=18) as recp,
            tc.tile_pool(name="osb", bufs=1) as osb,
            tc.tile_pool(name="ps_big", bufs=2, space="PSUM") as ps_big,
            tc.tile_pool(name="ps_acc", bufs=1, space="PSUM") as ps_acc,
        ):
            # ---- constants ----
            w_sb = {}
            b_sb = {}
            for t in ("q", "k", "v"):
                w_sb[t] = const.tile([P, DC, P], BF16, tag=f"w{t}", name=f"w{t}")
                # [D, P] -> [p, c, e]; cast f32->bf16 inline (SWDGE)
                nc.gpsimd.dma_start(
                    w_sb[t][:],
                    dr_w[t].ap().rearrange("(c p) e -> p c e", p=P))
                b_sb[t] = const.tile([P, 1], F32, tag=f"b{t}", name=f"b{t}")
                nc.sync.dma_start(
                    b_sb[t][:],
                    dr_b[t].ap().rearrange("(p o) -> p o", o=1))
            ident = const.tile([P, P], BF16, tag="ident", name="ident")
            make_identity(nc, ident[:])

            def proj_dbl_chunk(t, b, cc, sinks):
                """Load a 2MB double D-chunk (two 128-row slabs in one
                dma_start for better DMA efficiency) and run its
                projection matmuls.  sinks(c, rhs_slice_fn) emits them."""
                x = stream.tile([P, 2, S], BF16, tag="stream", name="x")
                nc.gpsimd.dma_start(
                    x[:],
                    dr_in[t].ap()[b, cc * 2 * P:(cc + 1) * 2 * P, :]
                    .rearrange("(two p) s -> p two s", two=2))
                for two in range(2):
                    sinks(cc * 2 + two, x[:, two, :])

            def emit_qt(b, t="q", tag="qT"):
                """Q/K projection: 4 double-chunks -> [d, S] bf16."""
                halves = [ps_big.tile([P, 1024], F32, tag="big",
                                      name="q_ps") for _ in range(2)]

                def sinks(c, rhs):
                    for h in range(2):
                        for s2 in range(2):
                            nc.tensor.matmul(
                                halves[h][:, s2 * 512:(s2 + 1) * 512],
                                lhsT=w_sb[t][:, c, :],
                                rhs=rhs[:, h * 1024 + s2 * 512:
                                        h * 1024 + (s2 + 1) * 512],
                                start=(c == 0), stop=(c == DC - 1))

                for cc in range(DC // 2):
                    proj_dbl_chunk(t, b, cc, sinks)
                out = proj.tile([P, S], BF16, tag=tag, name=tag)
                for h in range(2):
                    nc.vector.tensor_scalar_add(
                        out[:, h * 1024:(h + 1) * 1024],
                        halves[h][:], b_sb[t][:])
                return out

            def emit_vt_chunk(b, vt_ps, cc):
                """One double D-chunk of the V projection ("acc")."""

                def sinks(c, rhs):
                    for r in range(4):
                        nc.tensor.matmul(
                            vt_ps[:, r * 512:(r + 1) * 512],
                            lhsT=w_sb["v"][:, c, :],
                            rhs=rhs[:, r * 512:(r + 1) * 512],
                            start=(c == 0), stop=(c == DC - 1))

                proj_dbl_chunk("v", b, cc, sinks)

            def emit_v_finish(vt_ps):
                """Bias-add, PE-transpose VT -> V [S, d]."""
                vt_sb = proj.tile([P, S], BF16, tag="vT", name="vT")
                for h in range(2):
                    nc.vector.tensor_scalar_add(
                        vt_sb[:, h * 1024:(h + 1) * 1024],
                        vt_ps[:, h * 1024:(h + 1) * 1024], b_sb["v"][:])
                v_sb = proj.tile([P, KC, P], BF16, tag="v", name="v")
                trt = ps_acc.tile([P, S], BF16, tag="acc", name="tr_ps")
                for idx in range(KC):
                    nc.tensor.transpose(
                        trt[:, idx * P:(idx + 1) * P],
                        vt_sb[:, idx * P:(idx + 1) * P], ident[:])
                for g in range(2):
                    nc.vector.tensor_copy(
                        v_sb[:, g * 8:(g + 1) * 8, :],
                        trt[:, g * 1024:(g + 1) * 1024].rearrange(
                            "p (a e) -> p a e", a=8))
                return v_sb

            def emit_kslab(b, sl):
                """K super-chunk: one [D, 256] slab -> kct [d, 256] bf16
                (2 k-chunks worth of KT), so scores start on the first
                slab instead of after the whole K projection.  The slab
                accumulator lives in the "acc" PSUM rotation, which is
                idle during the scores chain — the sc double-buffer in
                "big" stays undisturbed."""
                xk = stream.tile([P, DC, 256], BF16, tag="stream",
                                 name="xk")
                nc.gpsimd.dma_start(
                    xk[:],
                    dr_in["k"].ap()[b, :, sl * 256:(sl + 1) * 256]
                    .rearrange("(c p) s -> p c s", p=P))
                kps = ps_acc.tile([P, 256], F32, tag="acc", name="kps")
                for c in range(DC):
                    nc.tensor.matmul(
                        kps[:], lhsT=w_sb["k"][:, c, :], rhs=xk[:, c, :],
                        start=(c == 0), stop=(c == DC - 1))
                kct = kctp.tile([P, 256], BF16, tag="kt", name="kct")
                nc.vector.tensor_scalar_add(kct[:], kps[:], b_sb["k"][:])
                return kct

            def emit_scores(qt, lhsT_ap):
                """One k-chunk of scores^T + exp + 1/Z."""
                pt = ptp.tile([P, S], BF16, tag="pt", name="pt")
                zz = stats.tile([P, 2], F32, tag="z", name="zz")
                for h in range(2):
                    sc = ps_big.tile([P, 1024], F32, tag="big",
                                     name="sc_ps")
                    for s2 in range(2):
                        nc.tensor.matmul(
                            sc[:, s2 * 512:(s2 + 1) * 512],
                            lhsT=lhsT_ap,
                            rhs=qt[:, h * 1024 + s2 * 512:
                                   h * 1024 + (s2 + 1) * 512],
                            start=True, stop=True)
                    nc.scalar.activation(
                        pt[:, h * 1024:(h + 1) * 1024], sc[:],
                        func=mybir.ActivationFunctionType.Exp,
                        scale=SCALE, accum_out=zz[:, h:h + 1])
                rec = recp.tile([P, 1], F32, tag="rec", name="rec")
                nc.vector.tensor_reduce(
                    rec[:], zz[:], axis=mybir.AxisListType.X,
                    op=mybir.AluOpType.add)
                nc.vector.reciprocal(rec[:], rec[:])
                return pt, rec

            def emit_ht_mm(ht, v_sb, pt, rec, kc):
                vs = stats.tile([P, P], BF16, tag="vs", name="vs")
                nc.vector.tensor_scalar_mul(vs[:], v_sb[:, kc, :], rec[:])
                for st in range(4):
                    nc.tensor.matmul(
                        ht[:, st * 512:(st + 1) * 512],
                        lhsT=vs[:],
                        rhs=pt[:, st * 512:(st + 1) * 512],
                        start=(kc == 0), stop=(kc == KC - 1))

            def emit_out(b, ht):
                out_sb = osb.tile([P, S], F32, tag="osb", name="out_sb")
                nc.vector.tensor_copy(out_sb[:], ht[:])
                nc.sync.dma_start(dr_out.ap()[b], out_sb[:])

            for b in range(B_LOC):
                qt = emit_qt(b, "q", "qT")

                # K in [D, 256] slabs fused with the scores/exp chain:
                # exp starts on the first slab (right after q is loaded)
                # instead of after the whole K projection.
                pts = []
                recs = []
                for sl in range(8):
                    kct = emit_kslab(b, sl)
                    for j in range(2):
                        pt, rec = emit_scores(
                            qt, kct[:, j * P:(j + 1) * P])
                        pts.append(pt)
                        recs.append(rec)

                # V projection (v-DMAs follow the k slabs; the "acc"
                # rotation continues kps -> vt_ps -> trt -> ht)
                vt_ps = ps_acc.tile([P, S], F32, tag="acc", name="vt_ps")
                for cc in range(DC // 2):
                    emit_vt_chunk(b, vt_ps, cc)
                v_sb = emit_v_finish(vt_ps)
                ht = ps_acc.tile([P, S], F32, tag="acc", name="ht")
                for kc in range(KC):
                    emit_ht_mm(ht, v_sb, pts[kc], recs[kc], kc)
                emit_out(b, ht)

    nc.compile()
    return nc


def _get_nc():
    global _BUILT
    if _BUILT is None:
        _BUILT = build()
    return _BUILT


def kernel(inp_q, inp_k, inp_v, Wq_kernel, Wq_bias, Wk_kernel, Wk_bias,
           Wv_kernel, Wv_bias):
    from concourse.bass_utils import run_bass_kernel_spmd

    nc = _get_nc()

    inp = {"q": np.asarray(inp_q, dtype=np.float32),
           "k": np.asarray(inp_k, dtype=np.float32),
           "v": np.asarray(inp_v, dtype=np.float32)}
    w = {"q": np.ascontiguousarray(np.asarray(Wq_kernel, dtype=np.float32)),
         "k": np.ascontiguousarray(np.asarray(Wk_kernel, dtype=np.float32)),
         "v": np.ascontiguousarray(np.asarray(Wv_kernel, dtype=np.float32))}
    bias = {"q": np.ascontiguousarray(np.asarray(Wq_bias, dtype=np.float32)),
            "k": np.ascontiguousarray(np.asarray(Wk_bias, dtype=np.float32)),
            "v": np.ascontiguousarray(np.asarray(Wv_bias, dtype=np.float32))}

    in_maps = []
    for c in range(N_CORES):
        m = {}
        for t in ("q", "k", "v"):
            # [2, S, D] -> [2, D, S] contiguous (pure layout marshalling)
            m[f"{t}T"] = np.ascontiguousarray(
                inp[t][c * B_LOC:(c + 1) * B_LOC].transpose(0, 2, 1))
            m[f"w{t}"] = w[t]
            m[f"b{t}"] = bias[t]
        in_maps.append(m)

    res = run_bass_kernel_spmd(nc, in_maps, list(range(N_CORES)))

    out = np.empty((N_CORES * B_LOC, S, P), dtype=np.float32)
    for c in range(N_CORES):
        # [2, P, S] -> [2, S, P]
        out[c * B_LOC:(c + 1) * B_LOC] = (
            res.results[c]["out"].transpose(0, 2, 1))
    return out
